# revision 11
# baseline (speedup 1.0000x reference)
"""DMGCGRUCell Trainium2 kernel: 8-core SPMD (4 batch-groups x 2 s-halves).

Layout notes:
- t axis (region node contraction dim) is padded/split: [500 real | 12 pad | 500 real | 12 pad] = 1024.
- s axis (per-core output rows) is one 500-half padded to 512.
- A is host-normalized: deg_s * (A + I) * deg_t, sliced per s-half, t pad-split.
- All feature-major ("T") tensors are host-pretransposed.
"""
import numpy as np
import concourse.bass as bass
import concourse.tile as tile
from concourse import bacc, mybir
from concourse.bass_utils import run_bass_kernel_spmd

B, N, R, S, G = 16, 10000, 10, 1000, 4
DIN, DH = 32, 64
NCORES, CB, CS = 8, 4, 2
BL = B // CB          # 4 local batches
SH = S // CS          # 500 real rows per half
SCP = 512             # padded s-half
TP = 1024             # padded/split t
NTC = TP // 128       # 8 t-chunks
F32 = mybir.dt.float32
F32R = mybir.dt.float32r
AF = mybir.ActivationFunctionType
ALU = mybir.AluOpType
BIAS_W = np.array([0.1, 0.1, 0.1, 1.0], dtype=np.float32)

_cache = {}


def _build():
    nc = bacc.Bacc("TRN2", target_bir_lowering=False, debug=False, num_devices=NCORES)
    dt = F32R

    def din(name, shape):
        return nc.dram_tensor(name, list(shape), dt, kind="ExternalInput").ap()

    xT = din("xT", (BL, R, 32, TP))
    hT = din("hT", (BL, R, 64, TP))
    hpT = din("hpT", (BL, R, 64, SCP))
    rsT = din("rsT", (BL, R, 2, SCP))
    An = din("An", (G, R, SCP, TP))
    Wur = din("Wur", (96, 512))
    Wc = din("Wc", (96, 256))
    a1w = {k: din(f"a1w_{k}", (258, 64)) for k in "urc"}
    a1b = {k: din(f"a1b_{k}", (64, 1)) for k in "urc"}
    a2wb = {k: din(f"a2wb_{k}", (66, 4)) for k in "urc"}
    ident = din("ident", (128, 128))
    ones4 = din("ones4", (4, 1))
    ones14 = din("ones14", (1, 4))
    ones164 = din("ones164", (1, 64))
    sel4 = din("sel4", (4, 256))
    ones512 = din("ones512", (1, 512))
    out_l = nc.dram_tensor("out_l", [BL, R, SH, 64], dt, kind="ExternalOutput").ap()

    with tile.TileContext(nc, trace_sim=False) as tc:
        import contextlib
        ctx = contextlib.ExitStack()
        with ctx, nc.allow_low_precision(reason="f32r data tiles; matmul accumulation stays in f32 PSUM"):
            sb = ctx.enter_context(tc.tile_pool(name="sb", bufs=1))
            sb2 = ctx.enter_context(tc.tile_pool(name="sb2", bufs=2))
            const = ctx.enter_context(tc.tile_pool(name="const", bufs=1))
            ps_mm1 = ctx.enter_context(tc.tile_pool(name="ps_mm1", bufs=2, space="PSUM"))
            ps_tps = ctx.enter_context(tc.tile_pool(name="ps_tps", bufs=2, space="PSUM"))
            ps_mm2 = ctx.enter_context(tc.tile_pool(name="ps_mm2", bufs=2, space="PSUM"))

            dram = ctx.enter_context(tc.tile_pool(name="dram", bufs=1, space="DRAM"))

            # ---- constants
            id_t = const.tile([128, 128], dt, tag="ident")
            nc.sync.dma_start(id_t[:], ident[:])
            wur_t = const.tile([96, 512], dt, tag="wur")
            nc.sync.dma_start(wur_t[:], Wur[:])
            wc_t = const.tile([96, 256], dt, tag="wc")
            nc.sync.dma_start(wc_t[:], Wc[:])
            o4_t = const.tile([4, 1], dt, tag="o4")
            nc.sync.dma_start(o4_t[:], ones4[:])
            o14_t = const.tile([1, 4], dt, tag="o14")
            nc.sync.dma_start(o14_t[:], ones14[:])
            o164_t = const.tile([1, 64], dt, tag="o164")
            nc.sync.dma_start(o164_t[:], ones164[:])
            sel_t = const.tile([4, 256], dt, tag="sel4")
            nc.sync.dma_start(sel_t[:], sel4[:])
            a1w_t, a1b_t, a2wb_t = {}, {}, {}
            for k in "urc":
                a1w_t[k] = []
                for ci, (r0, r1) in enumerate(((0, 128), (128, 256), (256, 258))):
                    w = const.tile([r1 - r0, 64], dt, tag=f"a1w{k}{ci}", name=f"a1w{k}{ci}")
                    nc.sync.dma_start(w[:], a1w[k][r0:r1, :])
                    a1w_t[k].append(w)
                a1b_t[k] = const.tile([64, 1], dt, tag=f"a1b{k}", name=f"a1b{k}")
                nc.sync.dma_start(a1b_t[k][:], a1b[k][:])
                a2wb_t[k] = const.tile([66, 4], dt, tag=f"a2wb{k}", name=f"a2wb{k}")
                nc.sync.dma_start(a2wb_t[k][:], a2wb[k][:])

            u_scr = dram.tile([BL, R, 64, SCP], dt, tag="u_scr")
            rh_loc = dram.tile([BL, R, 64, SCP], dt, tag="rh_loc")
            rh_all = dram.tile([CS, BL, R, 64, SCP], dt, tag="rh_all")

            def mk_inpT(r, src_hi, hi_name):
                """[96, TP] input-transposed tiles per local batch."""
                tiles = []
                for b in range(BL):
                    t = sb.tile([96, TP], dt, tag=f"inpT{b}")
                    nc.sync.dma_start(t[0:32, :], xT[b, r])
                    if hi_name == "h":
                        nc.sync.dma_start(t[32:96, :], src_hi[b, r])
                    else:  # rh gathered: two halves
                        nc.sync.dma_start(t[32:96, 0:SCP], src_hi[0, b, r])
                        nc.sync.dma_start(t[32:96, SCP:TP], src_hi[1, b, r])
                    tiles.append(t)
                return tiles

            def mk_hw(r, inpT, w_tile, ncols, blocks):
                """mm1: hw[tc] tiles [128, G*BL*64] cols (g,b,e) per block."""
                hw = {blk: [sb.tile([128, 1024], dt, tag=f"hw{bi}{tcd}", name=f"hw{bi}{tcd}") for tcd in range(NTC)]
                      for bi, blk in enumerate(blocks)}
                for b in range(BL):
                    for tcd in range(NTC):
                        ps = ps_mm1.tile([128, ncols], F32, tag="mm1")
                        nc.tensor.matmul(ps[:], inpT[b][:, tcd * 128:(tcd + 1) * 128],
                                         w_tile[:], start=True, stop=True)
                        for i, blk in enumerate(blocks):
                            src = ps[:, i * 256:(i + 1) * 256].rearrange("p (g e) -> p g e", g=G)
                            dst = hw[blk][tcd][:, :].rearrange("p (g b e) -> p g b e", g=G, b=BL)[:, :, b]
                            nc.any.tensor_copy(dst, src)
                return hw

            def mk_ATd(g, r):
                """transpose An[g,r] -> ATd[tc] [128 t, 512 s] tiles."""
                anat = []
                for m in range(4):
                    a = sb2.tile([128, TP], dt, tag=f"anat{m}", name=f"anat{m}", bufs=1)
                    nc.sync.dma_start(a[:], An[g, r, m * 128:(m + 1) * 128, :])
                    anat.append(a)
                ATd = []
                for tcd in range(NTC):
                    tps = ps_tps.tile([128, 512], dt, tag="tps")
                    for m in range(4):
                        nc.tensor.transpose(tps[:, m * 128:(m + 1) * 128],
                                            anat[m][:, tcd * 128:(tcd + 1) * 128], id_t[:])
                    at = sb.tile([128, 512], dt, tag=f"at{tcd}")
                    nc.any.tensor_copy(at[:], tps[:])
                    ATd.append(at)
                return ATd

            def mm2(g, ATd, hw_blk, HT_blk):
                """out2T accumulate + relu into HT tiles."""
                pss = []
                for m2 in range(2):
                    ps = ps_mm2.tile([128, 512], F32, tag=f"mm2_{m2}")
                    pss.append(ps)
                for tcd in range(NTC):
                    for m2 in range(2):
                        nc.tensor.matmul(pss[m2][:],
                                         hw_blk[tcd][:, g * 256 + m2 * 128: g * 256 + (m2 + 1) * 128],
                                         ATd[tcd][:], start=(tcd == 0), stop=(tcd == NTC - 1))
                lohi, off = g // 2, (g % 2) * 64
                for m2 in range(2):
                    for i in range(2):
                        b = 2 * m2 + i
                        nc.scalar.activation(HT_blk[b][lohi][off:off + 64, :],
                                             pss[m2][i * 64:(i + 1) * 64, :], AF.Relu)

            def attention(b, r, blk, HT_b, rsT_t, rs1_t):
                """T-layout attention; returns acc [64, SCP] combined output."""
                zps = ps_mm1.tile([64, 512], F32, tag="mm1")
                nc.tensor.matmul(zps[:], a1w_t[blk][0][:], HT_b[0][:], start=True, stop=False)
                nc.tensor.matmul(zps[:], a1w_t[blk][1][:], HT_b[1][:], start=False, stop=False)
                nc.tensor.matmul(zps[:], a1w_t[blk][2][:], rsT_t[:], start=False, stop=True)
                zS = sb2.tile([66, 512], dt, tag="zS")
                nc.scalar.activation(zS[0:64, :], zps[:], AF.Relu, bias=a1b_t[blk][:])
                nc.vector.tensor_scalar(zS[64:65, :], rs1_t[:], 0.5, None, op0=ALU.is_gt)
                nc.sync.dma_start(zS[65:66, :], ones512[:])
                lg = ps_mm1.tile([4, 512], F32, tag="mm1")
                nc.tensor.matmul(lg[:], a2wb_t[blk][:], zS[:], start=True, stop=True)
                aU = sb2.tile([4, 512], dt, tag="aU", bufs=1)
                nc.scalar.activation(aU[:], lg[:], AF.Exp)
                sm = ps_mm1.tile([1, 512], F32, tag="mm1")
                nc.tensor.matmul(sm[:], o4_t[:], aU[:], start=True, stop=True)
                rec = sb2.tile([1, 512], dt, tag="rec", bufs=1)
                nc.vector.reciprocal(rec[:], sm[:])
                rb4 = ps_mm1.tile([4, 512], F32, tag="mm1")
                nc.tensor.matmul(rb4[:], o14_t[:], rec[:], start=True, stop=True)
                aN = sb2.tile([4, 512], dt, tag="aN", bufs=1)
                nc.vector.tensor_mul(aN[:], aU[:], rb4[:])
                acc = sb2.tile([64, 512], dt, tag="acc")
                tmp = sb2.tile([64, 512], dt, tag="tmp")
                for g in range(G):
                    ab = ps_mm1.tile([64, 512], F32, tag="mm1")
                    nc.tensor.matmul(ab[:], sel_t[:, g * 64:(g + 1) * 64], aN[:], start=True, stop=True)
                    src = HT_b[g // 2][(g % 2) * 64:(g % 2) * 64 + 64, :]
                    if g == 0:
                        nc.vector.tensor_mul(acc[:], src, ab[:])
                    else:
                        nc.vector.tensor_mul(tmp[:], src, ab[:])
                        nc.vector.tensor_add(acc[:], acc[:], tmp[:])
                return acc

            # ================= PASS 1: blocks u, r =================
            for r in range(R):
                inpT = mk_inpT(r, hT, "h")
                hw = mk_hw(r, inpT, wur_t, 512, ("u", "r"))
                HT = {blk: [[sb.tile([128, 512], dt, tag=f"HT{bi}{b}{lh}", name=f"HT{bi}{b}{lh}") for lh in range(2)]
                            for b in range(BL)] for bi, blk in enumerate(("u", "r"))}
                for g in range(G):
                    ATd = mk_ATd(g, r)
                    for blk in ("u", "r"):
                        mm2(g, ATd, hw[blk], HT[blk])
                for b in range(BL):
                    rsT_t = sb2.tile([2, 512], dt, tag="rsT")
                    nc.sync.dma_start(rsT_t[:], rsT[b, r])
                    rs1_t = sb2.tile([1, 512], dt, tag="rs1")
                    nc.sync.dma_start(rs1_t[:], rsT[b, r, 1:2, :])
                    accu = attention(b, r, "u", HT["u"][b], rsT_t, rs1_t)
                    uT = sb2.tile([64, 512], dt, tag="uT")
                    nc.scalar.activation(uT[:], accu[:], AF.Sigmoid)
                    nc.sync.dma_start(u_scr[b, r], uT[:])
                    accr = attention(b, r, "r", HT["r"][b], rsT_t, rs1_t)
                    hp_t = sb2.tile([64, 512], dt, tag="hp")
                    nc.sync.dma_start(hp_t[:], hpT[b, r])
                    rh = sb2.tile([64, 512], dt, tag="rh")
                    nc.vector.tensor_mul(rh[:], accr[:], hp_t[:])
                    nc.sync.dma_start(rh_loc[b, r], rh[:])

            # ================= AllGather rh between s-half partners =================
            nc.gpsimd.collective_compute(
                "AllGather", ALU.bypass,
                replica_groups=[[0, 1], [2, 3], [4, 5], [6, 7]],
                ins=[rh_loc.opt()], outs=[rh_all.opt()])

            # ================= PASS 2: block c + GRU =================
            for r in range(R):
                candT = mk_inpT(r, rh_all, "rh")
                hw = mk_hw(r, candT, wc_t, 256, ("c",))
                HT = {"c": [[sb.tile([128, 512], dt, tag=f"HT0{b}{lh}", name=f"HT0{b}{lh}") for lh in range(2)]
                            for b in range(BL)]}
                for g in range(G):
                    ATd = mk_ATd(g, r)
                    mm2(g, ATd, hw["c"], HT["c"])
                for b in range(BL):
                    rsT_t = sb2.tile([2, 512], dt, tag="rsT")
                    nc.sync.dma_start(rsT_t[:], rsT[b, r])
                    rs1_t = sb2.tile([1, 512], dt, tag="rs1")
                    nc.sync.dma_start(rs1_t[:], rsT[b, r, 1:2, :])
                    acch = attention(b, r, "c", HT["c"][b], rsT_t, rs1_t)
                    th = sb2.tile([64, 512], dt, tag="th")
                    nc.scalar.activation(th[:], acch[:], AF.Tanh)
                    uT_t = sb2.tile([64, 512], dt, tag="uTl")
                    nc.sync.dma_start(uT_t[:], u_scr[b, r])
                    hp_t = sb2.tile([64, 512], dt, tag="hp")
                    nc.sync.dma_start(hp_t[:], hpT[b, r])
                    o1 = sb2.tile([64, 512], dt, tag="o1")
                    nc.vector.tensor_mul(o1[:], uT_t[:], th[:])
                    o2 = sb2.tile([64, 512], dt, tag="o2")
                    nc.vector.tensor_mul(o2[:], uT_t[:], hp_t[:])
                    nc.vector.tensor_sub(o1[:], o1[:], o2[:])
                    oT = o1
                    nc.vector.tensor_add(oT[:], oT[:], hp_t[:])
                    gps = ps_mm1.tile([128, 256], dt, tag="mm1")
                    for m in range(4):
                        nc.tensor.transpose(gps[:, m * 64:(m + 1) * 64],
                                            oT[:, m * 128:(m + 1) * 128], id_t[0:64, 0:64])
                    gs = sb2.tile([128, 256], dt, tag="gs")
                    nc.any.tensor_copy(gs[:], gps[:])
                    dst1 = out_l[b, r, 0:384, :].rearrange("(m p) e -> p m e", p=128)
                    src1 = gs[:, 0:192].rearrange("p (m e) -> p m e", m=3)
                    nc.sync.dma_start(dst1, src1)
                    nc.sync.dma_start(out_l[b, r, 384:500, :], gs[0:116, 192:256])

    nc.compile()
    return nc


def _prep(inputs):
    """Host-side shard + layout prep. Returns in_maps (len 8)."""
    A = np.asarray(inputs["A"], np.float32)
    deg = np.clip(A.sum(-1), 1e-5, None) ** -0.5          # [G,R,S]
    An_f = deg[..., :, None] * (A + np.eye(S, dtype=np.float32)) * deg[..., None, :]

    def padsplit_t(x):
        """[..., S] -> [..., TP] pad-split last axis."""
        out = np.zeros(x.shape[:-1] + (TP,), np.float32)
        out[..., 0:SH] = x[..., 0:SH]
        out[..., SCP:SCP + SH] = x[..., SH:S]
        return out

    An_ps = padsplit_t(An_f)                               # [G,R,S,TP]
    An_half = []
    for h in range(CS):
        a = np.zeros((G, R, SCP, TP), np.float32)
        a[:, :, 0:SH, :] = An_ps[:, :, h * SH:(h + 1) * SH, :]
        An_half.append(np.ascontiguousarray(a))

    x_t = np.asarray(inputs["x_t"], np.float32).reshape(B, R, S, DIN)
    h_prev = np.asarray(inputs["h_prev"], np.float32).reshape(B, R, S, DH)
    rs = np.asarray(inputs["resid_stats"], np.float32).reshape(B, R, S, 2)
    xT_f = padsplit_t(x_t.transpose(0, 1, 3, 2))           # [B,R,32,TP]
    hT_f = padsplit_t(h_prev.transpose(0, 1, 3, 2))        # [B,R,64,TP]
    rsT_f = np.zeros((B, R, 2, S + 2 * (SCP - SH)), np.float32)
    rsT_ps = padsplit_t(rs.transpose(0, 1, 3, 2))          # [B,R,2,TP]

    Wur = np.concatenate([inputs["W_u"].transpose(1, 0, 2).reshape(96, 256),
                          inputs["W_r"].transpose(1, 0, 2).reshape(96, 256)], axis=1)
    Wc = np.ascontiguousarray(inputs["W_c"].transpose(1, 0, 2).reshape(96, 256))
    log1p_bw = np.log1p(BIAS_W).reshape(1, 4)
    common = {
        "Wur": np.ascontiguousarray(Wur).astype(np.float32),
        "Wc": Wc.astype(np.float32),
        "ident": np.eye(128, dtype=np.float32),
        "ones4": np.ones((4, 1), np.float32),
        "ones14": np.ones((1, 4), np.float32),
        "ones164": np.ones((1, 64), np.float32),
        "sel4": np.kron(np.eye(4, dtype=np.float32), np.ones((1, 64), np.float32)),
        "ones512": np.ones((1, 512), np.float32),
    }
    for k in "urc":
        common[f"a1w_{k}"] = np.asarray(inputs[f"a1w_{k}"], np.float32)
        common[f"a1b_{k}"] = np.asarray(inputs[f"a1b_{k}"], np.float32).reshape(64, 1)
        common[f"a2wb_{k}"] = np.concatenate(
            [np.asarray(inputs[f"a2w_{k}"], np.float32), log1p_bw,
             np.asarray(inputs[f"a2b_{k}"], np.float32).reshape(1, 4)], axis=0)

    in_maps = []
    for core in range(NCORES):
        gb, sh = core // CS, core % CS
        bs = slice(gb * BL, (gb + 1) * BL)
        s0 = sh * SH
        hpT_c = np.zeros((BL, R, 64, SCP), np.float32)
        hpT_c[..., 0:SH] = h_prev[bs].transpose(0, 1, 3, 2)[..., s0:s0 + SH]
        rsT_c = np.zeros((BL, R, 2, SCP), np.float32)
        rsT_c[..., 0:SH] = rs[bs].transpose(0, 1, 3, 2)[..., s0:s0 + SH]
        m = dict(common)
        m["xT"] = np.ascontiguousarray(xT_f[bs])
        m["hT"] = np.ascontiguousarray(hT_f[bs])
        m["hpT"] = hpT_c
        m["rsT"] = rsT_c
        m["An"] = An_half[sh]
        in_maps.append(m)
    return in_maps


def kernel(**inputs) -> np.ndarray:
    if "nc" not in _cache:
        _cache["nc"] = _build()
    nc = _cache["nc"]
    in_maps = _prep(inputs)
    res = run_bass_kernel_spmd(nc, in_maps, list(range(NCORES)))
    out = np.zeros((B, R, S, DH), np.float32)
    for core in range(NCORES):
        gb, sh = core // CS, core % CS
        o = res.results[core]["out_l"]                     # [BL,R,SH,64]
        out[gb * BL:(gb + 1) * BL, :, sh * SH:(sh + 1) * SH, :] = o
    return out.reshape(B, N, DH)
